# revision 6
# baseline (speedup 1.0000x reference)
"""Multi-head attention (B=4, S=2048, D=1024, H=16) on 8 TRN2 NeuronCores.

Sharding: 8-way over (batch, head-half) — tensor parallel over heads.
Core c handles batch b=c//2 and heads hh*8..hh*8+8 (hh=c%2), ALL 2048
query rows. K/V are computed once per (batch, head) — no duplicated
projection FLOPs. The output projection is row-sharded over the concat
dim; the two partial outputs of a batch are summed ON THE HOST (the
"all-reduce"), together with the bo bias. PE streamed-column count per
core: V 65.5k + K 65.5k + Q 65.5k + scores 262k + PV 262k + out 65.5k
= 786.5k cols (~328 us at 2.4 GHz).

On-chip dataflow is fully "transposed" so no on-chip transposes are
needed (all matmuls bf16 with fp32 PSUM accumulation):
  V[kv, hdk]   = X_Tv^T Wv          (lhsT = xtv chunk, rhs = wv)
  Q_T[hdk, q]  = Wq_p^T X_T         (lhsT = wq tile,  rhs = xt)
  K_T[hdk, kv] = Wk_p^T X_T
  S_T[kv, q]   = K_T^T Q_T          (per head, K=64 contraction)
  p_T          = exp(0.125 * S_T) * m01  (exp on ACT psum->sbuf bf16,
                                     multiplicative 0/1 bf16 mask on DVE
                                     at the 2x bf16 rate)
  O_T[65, q]   = [V_h | 1]^T p_T    (lhsT = V augmented with a ones
                                     column; row 64 = softmax denom)
  concat_T     = O_T[0:64] * recip(O_T[64]) + bv
                 (denominator row bounced through DRAM into [128,8] so
                  the reciprocal uses all DVE lanes, broadcast back via
                  DMA; bias-add on the otherwise-idle GPSIMD engine)
  y_partial    = concat_T^T Wo^T    (row-sharded; host adds pair + bo)

Scheduling: V proj streams xtv in 1MB chunks so the xt/mask/weight DMAs
overlap it; Q/K projections for pair p+1 and the qh0 output projection
are drip-fed into the ACT/DVE-bound attention loop as PE filler work.
"""

import sys

if "/opt/trn_rl_repo" not in sys.path:
    sys.path.insert(0, "/opt/trn_rl_repo")

import numpy as np
import ml_dtypes

B, S, D, H = 4, 2048, 1024, 16
DK = D // H  # 64
NCORES = 8
HL = H // 2  # 8 local heads per core
NP = HL // 2  # 4 local head pairs
NDT = D // 128  # 8 d-tiles
NKV = S // 128  # 16 kv tiles
NQH = 2  # q halves (1024 each)
QH = S // NQH  # 1024
BF16 = ml_dtypes.bfloat16

_CACHE = {}


def _patch_tile_drain():
    """This walrus build rejects >1 sem-wait on the CTRL (drain) struct and
    wide sem-range clears; split the Tile tail-drain's waits and chunk the
    semaphore frees."""
    import concourse.tile as tile
    import concourse.mybir as mybir
    from concourse.vector_clock import ScopedClock

    if getattr(tile.TileContext, "_drain_split_patched", False):
        return

    def _drain_and_barrier(self, tick_clock, wait_clock):
        nc = self.nc
        drain_inst = nc.sync.drain()
        wait_clock.add_sem_waits(
            drain_inst.ins, ScopedClock({None: tick_clock.global_clock})
        )
        si = drain_inst.ins.sync_info
        if si is not None and len(si.on_wait) > 1:
            waits = list(si.on_wait)
            drain_inst.ins.sync_info = mybir.SyncInfo(
                on_wait=waits[:1], on_update=list(si.on_update)
            )
            for w in waits[1:]:
                extra = nc.sync.drain()
                extra.ins.sync_info = mybir.SyncInfo(on_wait=[w], on_update=[])
        nc.all_engine_barrier()
        popped = nc._tile_sem_poison_stack.pop()
        assert popped is self._sem_poison
        sems = sorted(
            self.sems.allocated().values(),
            key=lambda s: s.num if hasattr(s, "num") else s,
        )
        for i in range(0, len(sems), 3):
            nc.clear_and_free_semaphores(sems[i : i + 3])
        nc.all_engine_barrier()

    tile.TileContext._drain_and_barrier = _drain_and_barrier
    tile.TileContext._drain_split_patched = True


def _split_excess_waits(nc, max_waits=1):
    """Walrus (this build) rejects instructions with more than one sem-wait.
    Move overflow waits onto same-engine EventSemaphore instructions inserted
    just before the overloaded instruction (per-engine order preserved)."""
    import concourse.mybir as mybir

    n = 0
    for fn in nc.m.functions:
        for bb in fn.blocks:
            out = []
            changed = False
            for inst in bb.instructions:
                si = getattr(inst, "sync_info", None)
                waits = list(si.on_wait) if si is not None else []
                if len(waits) > max_waits:
                    for w in waits[:-max_waits]:
                        n += 1
                        ev = mybir.InstEventSemaphore(
                            name=f"WSPLIT-{n}", ins=[], outs=[]
                        )
                        ev.engine = inst.engine
                        ev.sync_info = mybir.SyncInfo(on_wait=[w], on_update=[])
                        out.append(ev)
                    inst.sync_info = mybir.SyncInfo(
                        on_wait=waits[-max_waits:], on_update=list(si.on_update)
                    )
                    changed = True
                out.append(inst)
            if changed:
                bb.instructions = out
    return n


def _build():
    """Build the single-core SPMD Bass program (same for all 8 cores)."""
    import concourse.bass as bass
    import concourse.tile as tile
    import concourse.mybir as mybir

    _patch_tile_drain()

    f32 = mybir.dt.float32
    bf16 = mybir.dt.bfloat16
    ACT = mybir.ActivationFunctionType

    nc = bass.Bass("TRN2", target_bir_lowering=False, debug=False)

    # ---- kernel I/O (per-core shards, host-prepped) ----
    # context transposed [D, S] (feeds both Q and K projections)
    xt = nc.dram_tensor("xt", [D, S], bf16, kind="ExternalInput").ap()
    # value transposed, chunked [super-chunk, d-tile, 128 d, 512 kv]
    xtv = nc.dram_tensor("xtv", [4, NDT, 128, 512], bf16, kind="ExternalInput").ap()
    # mask^T, 1.0 = KEEP (multiplicative, applied post-exp)
    mskt = nc.dram_tensor("mskt", [S, S], bf16, kind="ExternalInput").ap()
    # wq/wk: [pair, dtile, 128 d, 128 cols(2 heads x 64 dk)]
    wq = nc.dram_tensor("wq", [NP, NDT, 128, 128], bf16, kind="ExternalInput").ap()
    wk = nc.dram_tensor("wk", [NP, NDT, 128, 128], bf16, kind="ExternalInput").ap()
    # wv: [dtile, 128 d, 512 cols(8 local heads x 64)]
    wv = nc.dram_tensor("wv", [NDT, 128, 512], bf16, kind="ExternalInput").ap()
    # wot: Wo^T rows for our 512 concat dims, tiled [dtile, 128 din, 1024 dout]
    wot = nc.dram_tensor("wot", [NP, 128, D], bf16, kind="ExternalInput").ap()
    bq_t = nc.dram_tensor("bq_t", [128, NP], f32, kind="ExternalInput").ap()
    bk_t = nc.dram_tensor("bk_t", [128, NP], f32, kind="ExternalInput").ap()
    bv_t = nc.dram_tensor("bv_t", [DK, HL], f32, kind="ExternalInput").ap()
    # partial output (row-sharded out-proj contribution; host sums + bo)
    y = nc.dram_tensor("y", [S, D], f32, kind="ExternalOutput").ap()

    with tile.TileContext(nc) as tc:
        with (
            tc.tile_pool(name="persist", bufs=1) as persist,
            tc.tile_pool(name="mskp", bufs=1) as mskp,
            tc.tile_pool(name="xtp", bufs=1) as xtp,
            tc.tile_pool(name="wqk", bufs=2) as wqkp,
            tc.tile_pool(name="pexp", bufs=3) as pexp,
            tc.tile_pool(name="pmask", bufs=3) as pmask,
            tc.tile_pool(name="fin", bufs=1) as finp,
            tc.tile_pool(name="outsb", bufs=2) as outp,
            tc.tile_pool(name="psmain", bufs=3, space="PSUM") as psmain,
            tc.tile_pool(name="pso", bufs=1, space="PSUM") as pso,
            tc.tile_pool(name="dscr", bufs=2, space="DRAM") as dscr,
        ):
            # ---- persistent small tensors ----
            bq_sb = persist.tile([128, NP], f32, tag="bq")
            nc.sync.dma_start(bq_sb[:], bq_t[:])
            bk_sb = persist.tile([128, NP], f32, tag="bk")
            nc.sync.dma_start(bk_sb[:], bk_t[:])
            bv_sb = persist.tile([DK, HL], f32, tag="bv")
            nc.sync.dma_start(bv_sb[:], bv_t[:])

            # V augmented with a ones column per head: [128 kv, 2*65]
            # (col 64/129 = 1.0 -> PV matmul row 64 = softmax denominator)
            vaug_p = [
                persist.tile([128, NKV * 130], bf16, tag=f"va{p}", name=f"va{p}")
                for p in range(NP)
            ]

            def vaug(p, kv):
                return vaug_p[p][:, kv * 130 : (kv + 1) * 130]

            # concat_T: 4 din-tiles [128, S] (full q range)
            concat = [
                persist.tile([128, S], bf16, tag=f"cc{p}", name=f"cc{p}")
                for p in range(NP)
            ]
            # Q_T / K_T for all 4 pairs
            qt_all = [
                persist.tile([128, S], bf16, tag=f"qt{p}", name=f"qt{p}")
                for p in range(NP)
            ]
            kt_all = [
                persist.tile([128, S], bf16, tag=f"kt{p}", name=f"kt{p}")
                for p in range(NP)
            ]

            for p in range(NP):
                ones_ap = vaug_p[p].rearrange("a (k c) -> a k c", c=65)[:, :, 64:65]
                nc.gpsimd.memset(ones_ap, 1.0)

            # ---- phase 0: V projection (all local heads, value sequence) ----
            # xtv streamed in 1MB super-chunks so the xt / wq / mask DMAs can
            # run concurrently with the V-proj matmuls.
            with (
                tc.tile_pool(name="xtv", bufs=2) as xtvp,
                tc.tile_pool(name="wvp", bufs=1) as wvp,
            ):
                wv_sb = []
                for d in range(NDT):
                    t = wvp.tile([128, 512], bf16, tag=f"wv{d}", name=f"wv{d}")
                    nc.sync.dma_start(t[:], wv[d, :, :])
                    wv_sb.append(t)

                def load_xtv(sc):
                    t = xtvp.tile([128, NDT, 512], bf16, tag="xtv", name=f"xtv{sc}")
                    nc.sync.dma_start(t[:], xtv[sc].rearrange("t d c -> d t c"))
                    return t

                xtv_tiles = {0: load_xtv(0), 1: load_xtv(1)}

                # xt (context) loads — queued behind the first xtv chunks so
                # V-proj starts ASAP but xt still lands early.
                xt_sb = []
                for d in range(NDT):
                    t = xtp.tile([128, S], bf16, tag=f"xt{d}", name=f"xt{d}")
                    nc.sync.dma_start(t[:], xt[d * 128 : (d + 1) * 128, :])
                    xt_sb.append(t)

                # pair-0 projection weights, explicitly early (before masks)
                wq0_sb = wqkp.tile([128, NDT, 128], bf16, tag="wq", name="wq0")
                nc.sync.dma_start(wq0_sb[:], wq[0].rearrange("t d c -> d t c"))
                wk0_sb = wqkp.tile([128, NDT, 128], bf16, tag="wk", name="wk0")
                nc.sync.dma_start(wk0_sb[:], wk[0].rearrange("t d c -> d t c"))

                # mask tiles for qh0 (per-qh streamed, reloaded for qh1)
                msk_sb = {}

                def load_msk(qh, kv):
                    t = mskp.tile([128, QH], bf16, tag=f"m{kv}", name=f"m{qh}_{kv}")
                    nc.sync.dma_start(
                        t[:],
                        mskt[kv * 128 : (kv + 1) * 128, qh * QH : (qh + 1) * QH],
                    )
                    msk_sb[(qh, kv)] = t

                for kv in range(NKV):
                    load_msk(0, kv)

                # V-proj compute, streaming super-chunks
                for sc in range(4):
                    xtv_t = xtv_tiles.pop(sc)
                    if sc + 2 < 4:
                        xtv_tiles[sc + 2] = load_xtv(sc + 2)
                    for kvl in range(4):
                        kv = sc * 4 + kvl
                        ps_v = psmain.tile([128, 1024], f32, tag="ps")
                        for d in range(NDT):
                            nc.tensor.matmul(
                                ps_v[:, 0:512],
                                xtv_t[:, d, kvl * 128 : (kvl + 1) * 128],
                                wv_sb[d][:],
                                start=(d == 0),
                                stop=(d == NDT - 1),
                            )
                        for p in range(NP):
                            dst = vaug(p, kv).rearrange("a (h c) -> a h c", c=65)[
                                :, :, 0:64
                            ]
                            src = ps_v[:, p * 128 : (p + 1) * 128].rearrange(
                                "a (h c) -> a h c", c=64
                            )
                            nc.vector.tensor_copy(dst, src)

            # wot lands in the SBUF space freed by xtv/wv (needed late)
            with tc.tile_pool(name="wot", bufs=1) as wotp:
                wot_sb = []
                for d in range(NP):
                    t = wotp.tile([128, D], bf16, tag=f"wot{d}", name=f"wot{d}")
                    nc.sync.dma_start(t[:], wot[d, :, :])
                    wot_sb.append(t)

                def proj_steps(p, wq_sb=None, wk_sb=None):
                    """Generator of small closures; each emits ~2 PE matmuls
                    (or a DMA / bias-copy). Together they produce
                    qt_all[p] / kt_all[p]."""
                    state = {}

                    def dma_w():
                        if wq_sb is None:
                            t = wqkp.tile(
                                [128, NDT, 128], bf16, tag="wq", name=f"wq{p}"
                            )
                            nc.sync.dma_start(t[:], wq[p].rearrange("t d c -> d t c"))
                            state["wq"] = t
                            t = wqkp.tile(
                                [128, NDT, 128], bf16, tag="wk", name=f"wk{p}"
                            )
                            nc.sync.dma_start(t[:], wk[p].rearrange("t d c -> d t c"))
                            state["wk"] = t
                        else:
                            state["wq"], state["wk"] = wq_sb, wk_sb

                    yield dma_w

                    # Q projection: out [128 hdk, S]; two 1024-col psum tiles
                    for qh in range(NQH):

                        def q_alloc(qh=qh):
                            state["psq"] = psmain.tile(
                                [128, 1024], f32, tag="ps", name=f"pjq{p}_{qh}"
                            )

                        yield q_alloc
                        for ch in range(2):
                            for d0 in range(0, NDT, 2):

                                def q_mm(qh=qh, ch=ch, d0=d0):
                                    cs = qh * 1024 + ch * 512
                                    for d in (d0, d0 + 1):
                                        nc.tensor.matmul(
                                            state["psq"][:, ch * 512 : (ch + 1) * 512],
                                            state["wq"][:, d, :],
                                            xt_sb[d][:, cs : cs + 512],
                                            start=(d == 0),
                                            stop=(d == NDT - 1),
                                        )

                                yield q_mm

                        def q_copy(qh=qh):
                            nc.vector.tensor_scalar_add(
                                qt_all[p][:, qh * 1024 : (qh + 1) * 1024],
                                state["psq"][:],
                                bq_sb[:, p : p + 1],
                            )

                        yield q_copy

                    # K projection: out [128 hdk, S]
                    for half in range(2):

                        def k_alloc(half=half):
                            state["psk"] = psmain.tile(
                                [128, 1024], f32, tag="ps", name=f"pjk{p}_{half}"
                            )

                        yield k_alloc
                        for ch in range(2):
                            for d0 in range(0, NDT, 2):

                                def k_mm(half=half, ch=ch, d0=d0):
                                    cs = half * 1024 + ch * 512
                                    for d in (d0, d0 + 1):
                                        nc.tensor.matmul(
                                            state["psk"][:, ch * 512 : (ch + 1) * 512],
                                            state["wk"][:, d, :],
                                            xt_sb[d][:, cs : cs + 512],
                                            start=(d == 0),
                                            stop=(d == NDT - 1),
                                        )

                                yield k_mm

                        def k_copy(half=half):
                            nc.vector.tensor_scalar_add(
                                kt_all[p][:, half * 1024 : (half + 1) * 1024],
                                state["psk"][:],
                                bk_sb[:, p : p + 1],
                            )

                        yield k_copy

                # run proj for pair 0 upfront (weights already in flight)
                for step in proj_steps(0, wq0_sb, wk0_sb):
                    step()

                from collections import deque

                work = deque()

                def out_proj_steps(qh):
                    """Output projection for q rows qh*1024..qh*1024+1024:
                    8 q-tiles x (2ch x 4d) matmuls + ACT copy + DMA."""
                    state = {}
                    for qt_l in range(8):
                        qt_i = qh * 8 + qt_l

                        def o_alloc(qt_i=qt_i):
                            state["psf"] = psmain.tile(
                                [128, 1024], f32, tag="ps", name=f"pso{qt_i}"
                            )

                        yield o_alloc
                        for ch in range(2):
                            for d0 in range(0, NP, 2):

                                def o_mm(qt_i=qt_i, ch=ch, d0=d0):
                                    qs = slice(qt_i * 128, (qt_i + 1) * 128)
                                    for d in (d0, d0 + 1):
                                        nc.tensor.matmul(
                                            state["psf"][:, ch * 512 : (ch + 1) * 512],
                                            concat[d][:, qs],
                                            wot_sb[d][:, ch * 512 : (ch + 1) * 512],
                                            start=(d == 0),
                                            stop=(d == NP - 1),
                                        )

                                yield o_mm

                        def o_out(qt_i=qt_i):
                            o_sb = outp.tile([128, 1024], f32, tag="out")
                            nc.scalar.copy(o_sb[:], state["psf"][:])
                            nc.sync.dma_start(
                                y[qt_i * 128 : (qt_i + 1) * 128, :], o_sb[:]
                            )

                        yield o_out

                # ---- attention: (qh, h, kv) with lag-1 PV + drip filler ----
                def emit_pv(p, lh, kv, ps_o, pm_t):
                    for ch in range(2):
                        chs = slice(ch * 512, (ch + 1) * 512)
                        nc.tensor.matmul(
                            ps_o[:, chs],
                            vaug(p, kv)[:, lh * 65 : (lh + 1) * 65],
                            pm_t[:, chs],
                            start=(kv == 0),
                            stop=(kv == NKV - 1),
                        )

                for qh in range(NQH):
                    if qh == 1:
                        work.extend(out_proj_steps(0))
                    for h in range(HL):
                        p, lh = h // 2, h % 2
                        if qh == 0 and h % 2 == 0 and h // 2 + 1 < NP:
                            work.extend(proj_steps(h // 2 + 1))
                        qt, kt = qt_all[p], kt_all[p]
                        hp = slice(lh * 64, (lh + 1) * 64)
                        ps_o = pso.tile([65, 1024], f32, tag="po", name=f"po{qh}_{h}")
                        prev_pm = None
                        for kv in range(NKV):
                            kvs = slice(kv * 128, (kv + 1) * 128)
                            ps_s = psmain.tile(
                                [128, 1024], f32, tag="ps", name=f"s{qh}_{h}_{kv}"
                            )
                            for ch in range(2):
                                cs = qh * 1024 + ch * 512
                                nc.tensor.matmul(
                                    ps_s[:, ch * 512 : (ch + 1) * 512],
                                    kt[hp, kvs],
                                    qt[hp, cs : cs + 512],
                                    start=True,
                                    stop=True,
                                )
                            if work:
                                work.popleft()()
                                if len(work) > 20 and work:
                                    work.popleft()()
                            pe_t = pexp.tile([128, 1024], bf16, tag="pe")
                            nc.scalar.activation(
                                pe_t[:], ps_s[:], ACT.Exp, scale=0.125
                            )
                            pm_t = pmask.tile([128, 1024], bf16, tag="pm")
                            nc.vector.tensor_mul(
                                pm_t[:], pe_t[:], msk_sb[(qh, kv)][:]
                            )
                            if qh == 0 and h == HL - 1:
                                # last qh0 reader of this mask tile: reload
                                # it with the qh1 slice (prefetch, WAR-safe)
                                load_msk(1, kv)
                            if prev_pm is not None:
                                emit_pv(p, lh, kv - 1, ps_o, prev_pm)
                            prev_pm = pm_t
                        emit_pv(p, lh, NKV - 1, ps_o, prev_pm)
                        # finalize (DVE/DMA/GPSIMD): copy PSUM out (frees the
                        # po slot), bounce the softmax sums through DRAM into
                        # [128, 8] so the reciprocal uses all DVE lanes, DMA
                        # the broadcast reciprocal back, multiply on DVE,
                        # bias-add on GPSIMD.
                        o_sb = finp.tile([65, 1024], f32, tag="osb", bufs=2)
                        nc.vector.tensor_copy(o_sb[:], ps_o[:])
                        dsum = dscr.tile([1024], f32, tag="dsum")
                        nc.sync.dma_start(
                            dsum.rearrange("(a b) -> a b", a=1), o_sb[64:65, :]
                        )
                        rs = finp.tile([128, 8], f32, tag="rs")
                        nc.sync.dma_start(rs[:], dsum.rearrange("(a b) -> a b", a=128))
                        rr = finp.tile([128, 8], f32, tag="rr")
                        nc.vector.reciprocal(rr[:], rs[:])
                        drec = dscr.tile([1024], f32, tag="drec")
                        nc.sync.dma_start(drec.rearrange("(a b) -> a b", a=128), rr[:])
                        rb = finp.tile([64, 1024], f32, tag="rb", bufs=2)
                        nc.sync.dma_start(
                            rb[:],
                            drec.rearrange("(a b) -> a b", a=1).partition_broadcast(64),
                        )
                        tmp = finp.tile([64, 1024], bf16, tag="tmp", bufs=2)
                        nc.vector.tensor_mul(tmp[:], o_sb[0:64, :], rb[:])
                        nc.gpsimd.tensor_scalar_add(
                            concat[p][hp, qh * 1024 : (qh + 1) * 1024],
                            tmp[:],
                            bv_sb[:, h : h + 1],
                        )
                while work:
                    work.popleft()()
                # ---- tail: output projection for qh1 ----
                for step in out_proj_steps(1):
                    step()

    _split_excess_waits(nc, max_waits=1)
    return nc


def _prep_inputs(context_sequence, value_sequence, mask, Wq, bq, Wk, bk, Wv, bv, Wo, bo):
    """Host-side shard prep: slice/transpose/cast per core."""
    ctx = np.asarray(context_sequence, dtype=np.float32)
    val = np.asarray(value_sequence, dtype=np.float32)
    mask = np.asarray(mask)
    Wq = np.asarray(Wq, dtype=np.float32)
    Wk = np.asarray(Wk, dtype=np.float32)
    Wv = np.asarray(Wv, dtype=np.float32)
    Wo = np.asarray(Wo, dtype=np.float32)
    bq = np.asarray(bq, dtype=np.float32)
    bk = np.asarray(bk, dtype=np.float32)
    bv = np.asarray(bv, dtype=np.float32)

    mskt = np.ascontiguousarray((mask == 0).T).astype(BF16)  # [S, S], 1.0=keep
    wo_t = np.ascontiguousarray(Wo.T)  # [din, dout]

    # per-batch transposed activations
    xt_b, xtv_b = [], []
    for b in range(B):
        xt_b.append(np.ascontiguousarray(ctx[b].T).astype(BF16))  # [D, S]
        xv = np.ascontiguousarray(val[b].T).astype(BF16)
        xtv_b.append(
            np.ascontiguousarray(
                xv.reshape(NDT, 128, 4, 512).transpose(2, 0, 1, 3)
            )
        )  # [4 sc, 8 d, 128, 512]

    # per-head-half weights
    def whalf(W, hh):  # [H, D, DK] -> [NP, NDT, 128, 128]
        Wf = W[hh * HL : (hh + 1) * HL].transpose(1, 0, 2).reshape(D, HL * DK)
        return np.ascontiguousarray(
            Wf.reshape(NDT, 128, NP, 128).transpose(2, 0, 1, 3)
        ).astype(BF16)

    shard = []
    for hh in range(2):
        g0 = hh * HL
        wq_t = whalf(Wq, hh)
        wk_t = whalf(Wk, hh)
        wv_t = np.ascontiguousarray(
            Wv[g0 : g0 + HL].transpose(1, 0, 2).reshape(D, HL * DK).reshape(
                NDT, 128, HL * DK
            )
        ).astype(BF16)
        wot_t = np.ascontiguousarray(
            wo_t[hh * 512 : (hh + 1) * 512, :].reshape(NP, 128, D)
        ).astype(BF16)
        bq_tt = np.ascontiguousarray(bq[g0 : g0 + HL].reshape(NP, 128).T)
        bk_tt = np.ascontiguousarray(bk[g0 : g0 + HL].reshape(NP, 128).T)
        bv_tt = np.ascontiguousarray(bv[g0 : g0 + HL].reshape(HL, DK).T)
        shard.append((wq_t, wk_t, wv_t, wot_t, bq_tt, bk_tt, bv_tt))

    in_maps = []
    for c in range(NCORES):
        b, hh = c // 2, c % 2
        wq_t, wk_t, wv_t, wot_t, bq_tt, bk_tt, bv_tt = shard[hh]
        in_maps.append(
            {
                "xt": xt_b[b],
                "xtv": xtv_b[b],
                "mskt": mskt,
                "wq": wq_t,
                "wk": wk_t,
                "wv": wv_t,
                "wot": wot_t,
                "bq_t": bq_tt,
                "bk_t": bk_tt,
                "bv_t": bv_tt,
            }
        )
    return in_maps


def _execute(inputs, trace=False):
    from concourse.bass_utils import run_bass_kernel_spmd

    if "nc" not in _CACHE:
        _CACHE["nc"] = _build()
    nc = _CACHE["nc"]
    in_maps = _prep_inputs(**inputs)
    res = run_bass_kernel_spmd(nc, in_maps, list(range(NCORES)), trace=trace)
    bo = np.asarray(inputs["bo"], dtype=np.float32)
    out = np.empty((B, S, D), dtype=np.float32)
    for b in range(B):
        out[b] = res.results[2 * b]["y"] + res.results[2 * b + 1]["y"] + bo[None, :]
    return out, res.exec_time_ns


def kernel(**inputs):
    out, _ = _execute(inputs, trace=False)
    return out


# revision 10
# speedup vs baseline: 1.3804x; 1.3804x over previous
"""Multi-head attention (B=4, S=2048, D=1024, H=16) on 8 TRN2 NeuronCores.

Sharding: 8-way over (batch, head-half) — tensor parallel over heads.
Core c handles batch b=c//2 and heads hh*8..hh*8+8 (hh=c%2), ALL 2048
query rows. K/V are computed once per (batch, head) — no duplicated
projection FLOPs. The output projection is row-sharded over the concat
dim; the two partial outputs of a batch are summed ON THE HOST (the
"all-reduce"), together with the bo bias. PE streamed-column count per
core: V 65.5k + K 65.5k + Q 65.5k + scores 262k + PV 262k + out 65.5k
= 786.5k cols (~328 us at 2.4 GHz).

On-chip dataflow is fully "transposed" so no on-chip transposes are
needed (all matmuls bf16 with fp32 PSUM accumulation):
  V[kv, hdk]   = X_Tv^T Wv          (lhsT = xtv chunk, rhs = wv)
  Q_T[hdk, q]  = Wq_p^T X_T         (lhsT = wq tile,  rhs = xt)
  K_T[hdk, kv] = Wk_p^T X_T
  S_T[kv, q]   = K_T^T Q_T          (per head, K=64 contraction)
  p_T          = exp(0.125 * S_T) * m01  (exp on ACT psum->sbuf bf16,
                                     multiplicative 0/1 bf16 mask on DVE
                                     at the 2x bf16 rate)
  O_T[65, q]   = [V_h | 1]^T p_T    (lhsT = V augmented with a ones
                                     column; row 64 = softmax denom)
  concat_T     = O_T[0:64] * recip(O_T[64]) + bv
                 (denominator row bounced through DRAM into [128,8] so
                  the reciprocal uses all DVE lanes, broadcast back via
                  DMA; bias-add on the otherwise-idle GPSIMD engine)
  y_partial    = concat_T^T Wo^T    (row-sharded; host adds pair + bo)

Scheduling: V proj streams xtv in 1MB chunks so the xt/mask/weight DMAs
overlap it; Q/K projections for pair p+1 and the qh0 output projection
are drip-fed into the ACT/DVE-bound attention loop as PE filler work.
"""

import sys

if "/opt/trn_rl_repo" not in sys.path:
    sys.path.insert(0, "/opt/trn_rl_repo")

import numpy as np
import ml_dtypes

B, S, D, H = 4, 2048, 1024, 16
DK = D // H  # 64
NCORES = 8
HL = H // 2  # 8 local heads per core
NP = HL // 2  # 4 local head pairs
NDT = D // 128  # 8 d-tiles
NKV = S // 128  # 16 kv tiles
NQH = 2  # q halves (1024 each)
QH = S // NQH  # 1024
BF16 = ml_dtypes.bfloat16

_CACHE = {}


def _patch_tile_drain():
    """This walrus build rejects >1 sem-wait on the CTRL (drain) struct and
    wide sem-range clears; split the Tile tail-drain's waits and chunk the
    semaphore frees."""
    import concourse.tile as tile
    import concourse.mybir as mybir
    from concourse.vector_clock import ScopedClock

    if getattr(tile.TileContext, "_drain_split_patched", False):
        return

    def _drain_and_barrier(self, tick_clock, wait_clock):
        nc = self.nc
        drain_inst = nc.sync.drain()
        wait_clock.add_sem_waits(
            drain_inst.ins, ScopedClock({None: tick_clock.global_clock})
        )
        si = drain_inst.ins.sync_info
        if si is not None and len(si.on_wait) > 1:
            waits = list(si.on_wait)
            drain_inst.ins.sync_info = mybir.SyncInfo(
                on_wait=waits[:1], on_update=list(si.on_update)
            )
            for w in waits[1:]:
                extra = nc.sync.drain()
                extra.ins.sync_info = mybir.SyncInfo(on_wait=[w], on_update=[])
        nc.all_engine_barrier()
        popped = nc._tile_sem_poison_stack.pop()
        assert popped is self._sem_poison
        sems = sorted(
            self.sems.allocated().values(),
            key=lambda s: s.num if hasattr(s, "num") else s,
        )
        for i in range(0, len(sems), 3):
            nc.clear_and_free_semaphores(sems[i : i + 3])
        nc.all_engine_barrier()

    tile.TileContext._drain_and_barrier = _drain_and_barrier
    tile.TileContext._drain_split_patched = True


def _split_excess_waits(nc, max_waits=1):
    """Walrus (this build) rejects instructions with more than one sem-wait.
    Move overflow waits onto same-engine EventSemaphore instructions inserted
    just before the overloaded instruction (per-engine order preserved)."""
    import concourse.mybir as mybir

    n = 0
    for fn in nc.m.functions:
        for bb in fn.blocks:
            out = []
            changed = False
            for inst in bb.instructions:
                si = getattr(inst, "sync_info", None)
                waits = list(si.on_wait) if si is not None else []
                if len(waits) > max_waits:
                    for w in waits[:-max_waits]:
                        n += 1
                        ev = mybir.InstEventSemaphore(
                            name=f"WSPLIT-{n}", ins=[], outs=[]
                        )
                        ev.engine = inst.engine
                        ev.sync_info = mybir.SyncInfo(on_wait=[w], on_update=[])
                        out.append(ev)
                    inst.sync_info = mybir.SyncInfo(
                        on_wait=waits[-max_waits:], on_update=list(si.on_update)
                    )
                    changed = True
                out.append(inst)
            if changed:
                bb.instructions = out
    return n


def _build():
    """Build the single-core SPMD Bass program (same for all 8 cores)."""
    import concourse.bass as bass
    import concourse.tile as tile
    import concourse.mybir as mybir

    _patch_tile_drain()

    f32 = mybir.dt.float32
    bf16 = mybir.dt.bfloat16
    ACT = mybir.ActivationFunctionType

    nc = bass.Bass("TRN2", target_bir_lowering=False, debug=False)

    # ---- kernel I/O (per-core shards, host-prepped) ----
    # context transposed [D, S] (feeds both Q and K projections)
    xt = nc.dram_tensor("xt", [D, S], bf16, kind="ExternalInput").ap()
    # value transposed, chunked [super-chunk, d-tile, 128 d, 512 kv]
    xtv = nc.dram_tensor("xtv", [4, NDT, 128, 512], bf16, kind="ExternalInput").ap()
    # mask^T, 1.0 = KEEP (multiplicative, applied post-exp)
    mskt = nc.dram_tensor("mskt", [S, S], bf16, kind="ExternalInput").ap()
    # wq/wk: [pair, dtile, 128 d, 128 cols(2 heads x 64 dk)]
    wq = nc.dram_tensor("wq", [NP, NDT, 128, 128], bf16, kind="ExternalInput").ap()
    wk = nc.dram_tensor("wk", [NP, NDT, 128, 128], bf16, kind="ExternalInput").ap()
    # wv: [dtile, 128 d, 512 cols(8 local heads x 64)]
    wv = nc.dram_tensor("wv", [NDT, 128, 512], bf16, kind="ExternalInput").ap()
    # wot: Wo^T rows for our 512 concat dims, tiled [dtile, 128 din, 1024 dout]
    wot = nc.dram_tensor("wot", [NP, 128, D], bf16, kind="ExternalInput").ap()
    bq_t = nc.dram_tensor("bq_t", [128, NP], f32, kind="ExternalInput").ap()
    bk_t = nc.dram_tensor("bk_t", [128, NP], f32, kind="ExternalInput").ap()
    bv_t = nc.dram_tensor("bv_t", [DK, HL], f32, kind="ExternalInput").ap()
    # partial output (row-sharded out-proj contribution; host sums + bo)
    y = nc.dram_tensor("y", [S, D], f32, kind="ExternalOutput").ap()

    with tile.TileContext(nc) as tc:
        with (
            tc.tile_pool(name="persist", bufs=1) as persist,
            tc.tile_pool(name="mskp", bufs=1) as mskp,
            tc.tile_pool(name="xtp", bufs=1) as xtp,
            tc.tile_pool(name="wqk", bufs=2) as wqkp,
            tc.tile_pool(name="pexp", bufs=3) as pexp,
            tc.tile_pool(name="pmask", bufs=3) as pmask,
            tc.tile_pool(name="fin", bufs=1) as finp,
            tc.tile_pool(name="outsb", bufs=2) as outp,
            tc.tile_pool(name="psmain", bufs=3, space="PSUM") as psmain,
            tc.tile_pool(name="pso", bufs=1, space="PSUM") as pso,
            tc.tile_pool(name="dscr", bufs=2, space="DRAM") as dscr,
        ):
            # V augmented with a ones column per head: [128 kv, 2*65]
            # (col 64/129 = 1.0 -> PV matmul row 64 = softmax denominator)
            vaug_p = [
                persist.tile([128, NKV * 130], bf16, tag=f"va{p}", name=f"va{p}")
                for p in range(NP)
            ]

            def vaug(p, kv):
                return vaug_p[p][:, kv * 130 : (kv + 1) * 130]

            # concat_T: 4 din-tiles [128, S] (full q range)
            concat = [
                persist.tile([128, S], bf16, tag=f"cc{p}", name=f"cc{p}")
                for p in range(NP)
            ]
            # Q_T / K_T for all 4 pairs
            qt_all = [
                persist.tile([128, S], bf16, tag=f"qt{p}", name=f"qt{p}")
                for p in range(NP)
            ]
            kt_all = [
                persist.tile([128, S], bf16, tag=f"kt{p}", name=f"kt{p}")
                for p in range(NP)
            ]

            for p in range(NP):
                ones_ap = vaug_p[p].rearrange("a (k c) -> a k c", c=65)[:, :, 64:65]
                nc.gpsimd.memset(ones_ap, 1.0)

            # ---- phase 0: V projection (all local heads, value sequence) ----
            # xtv streamed in 1MB super-chunks so the xt / wq / mask DMAs can
            # run concurrently with the V-proj matmuls.
            with (
                tc.tile_pool(name="xtv", bufs=2) as xtvp,
                tc.tile_pool(name="wvp", bufs=1) as wvp,
            ):
                # single combined DMA for all wv d-tiles (startup latency)
                wvc = wvp.tile([128, NDT, 512], bf16, tag="wv", name="wv")
                nc.sync.dma_start(wvc[:], wv.rearrange("t d c -> d t c"))
                wv_sb = [wvc[:, d, :] for d in range(NDT)]

                def load_xtv(sc):
                    t = xtvp.tile([128, NDT, 512], bf16, tag="xtv", name=f"xtv{sc}")
                    nc.sync.dma_start(t[:], xtv[sc].rearrange("t d c -> d t c"))
                    return t

                xtv_tiles = {0: load_xtv(0), 1: load_xtv(1)}

                # small bias tensors (needed from proj-p0 / finalize on)
                bq_sb = persist.tile([128, NP], f32, tag="bq")
                nc.sync.dma_start(bq_sb[:], bq_t[:])
                bk_sb = persist.tile([128, NP], f32, tag="bk")
                nc.sync.dma_start(bk_sb[:], bk_t[:])
                bv_sb = persist.tile([DK, HL], f32, tag="bv")
                nc.sync.dma_start(bv_sb[:], bv_t[:])

                # xt (context) loads — queued behind the first xtv chunks so
                # V-proj starts ASAP but xt still lands early.
                xt_sb = []
                for d in range(NDT):
                    t = xtp.tile([128, S], bf16, tag=f"xt{d}", name=f"xt{d}")
                    nc.sync.dma_start(t[:], xt[d * 128 : (d + 1) * 128, :])
                    xt_sb.append(t)

                # pair-0 projection weights, explicitly early (before masks)
                wq0_sb = wqkp.tile([128, NDT, 128], bf16, tag="wq", name="wq0")
                nc.sync.dma_start(wq0_sb[:], wq[0].rearrange("t d c -> d t c"))
                wk0_sb = wqkp.tile([128, NDT, 128], bf16, tag="wk", name="wk0")
                nc.sync.dma_start(wk0_sb[:], wk[0].rearrange("t d c -> d t c"))

                # mask tiles for qh0 (per-qh streamed, reloaded for qh1)
                msk_sb = {}

                def load_msk(qh, kv):
                    t = mskp.tile([128, QH], bf16, tag=f"m{kv}", name=f"m{qh}_{kv}")
                    nc.sync.dma_start(
                        t[:],
                        mskt[kv * 128 : (kv + 1) * 128, qh * QH : (qh + 1) * QH],
                    )
                    msk_sb[(qh, kv)] = t

                for kv in range(NKV):
                    load_msk(0, kv)

                # V-proj compute, streaming super-chunks
                for sc in range(4):
                    xtv_t = xtv_tiles.pop(sc)
                    if sc + 2 < 4:
                        xtv_tiles[sc + 2] = load_xtv(sc + 2)
                    for kvl in range(4):
                        kv = sc * 4 + kvl
                        ps_v = psmain.tile([128, 1024], f32, tag="ps")
                        for d in range(NDT):
                            nc.tensor.matmul(
                                ps_v[:, 0:512],
                                xtv_t[:, d, kvl * 128 : (kvl + 1) * 128],
                                wv_sb[d][:],
                                start=(d == 0),
                                stop=(d == NDT - 1),
                            )
                        for p in range(NP):
                            dst = vaug(p, kv).rearrange("a (h c) -> a h c", c=65)[
                                :, :, 0:64
                            ]
                            src = ps_v[:, p * 128 : (p + 1) * 128].rearrange(
                                "a (h c) -> a h c", c=64
                            )
                            nc.vector.tensor_copy(dst, src)

            # wot lands in the SBUF space freed by xtv/wv (needed late)
            with tc.tile_pool(name="wot", bufs=1) as wotp:
                wot_sb = []
                for d in range(NP):
                    t = wotp.tile([128, D], bf16, tag=f"wot{d}", name=f"wot{d}")
                    nc.sync.dma_start(t[:], wot[d, :, :])
                    wot_sb.append(t)

                def proj_steps(p, wq_sb=None, wk_sb=None):
                    """Generator of small closures; each emits ~2 PE matmuls
                    (or a DMA / bias-copy). Together they produce
                    qt_all[p] / kt_all[p]."""
                    state = {}

                    def dma_w():
                        if wq_sb is None:
                            t = wqkp.tile(
                                [128, NDT, 128], bf16, tag="wq", name=f"wq{p}"
                            )
                            nc.sync.dma_start(t[:], wq[p].rearrange("t d c -> d t c"))
                            state["wq"] = t
                            t = wqkp.tile(
                                [128, NDT, 128], bf16, tag="wk", name=f"wk{p}"
                            )
                            nc.sync.dma_start(t[:], wk[p].rearrange("t d c -> d t c"))
                            state["wk"] = t
                        else:
                            state["wq"], state["wk"] = wq_sb, wk_sb

                    yield dma_w

                    # Q projection: out [128 hdk, S]; two 1024-col psum tiles
                    for qh in range(NQH):

                        def q_alloc(qh=qh):
                            state["psq"] = psmain.tile(
                                [128, 1024], f32, tag="ps", name=f"pjq{p}_{qh}"
                            )

                        yield q_alloc
                        for ch in range(2):
                            for d0 in range(0, NDT, 2):

                                def q_mm(qh=qh, ch=ch, d0=d0):
                                    cs = qh * 1024 + ch * 512
                                    for d in (d0, d0 + 1):
                                        nc.tensor.matmul(
                                            state["psq"][:, ch * 512 : (ch + 1) * 512],
                                            state["wq"][:, d, :],
                                            xt_sb[d][:, cs : cs + 512],
                                            start=(d == 0),
                                            stop=(d == NDT - 1),
                                        )

                                yield q_mm

                        def q_copy(qh=qh):
                            nc.vector.tensor_scalar_add(
                                qt_all[p][:, qh * 1024 : (qh + 1) * 1024],
                                state["psq"][:],
                                bq_sb[:, p : p + 1],
                            )

                        yield q_copy

                    # K projection: out [128 hdk, S]
                    for half in range(2):

                        def k_alloc(half=half):
                            state["psk"] = psmain.tile(
                                [128, 1024], f32, tag="ps", name=f"pjk{p}_{half}"
                            )

                        yield k_alloc
                        for ch in range(2):
                            for d0 in range(0, NDT, 2):

                                def k_mm(half=half, ch=ch, d0=d0):
                                    cs = half * 1024 + ch * 512
                                    for d in (d0, d0 + 1):
                                        nc.tensor.matmul(
                                            state["psk"][:, ch * 512 : (ch + 1) * 512],
                                            state["wk"][:, d, :],
                                            xt_sb[d][:, cs : cs + 512],
                                            start=(d == 0),
                                            stop=(d == NDT - 1),
                                        )

                                yield k_mm

                        def k_copy(half=half):
                            nc.vector.tensor_scalar_add(
                                kt_all[p][:, half * 1024 : (half + 1) * 1024],
                                state["psk"][:],
                                bk_sb[:, p : p + 1],
                            )

                        yield k_copy

                # run proj for pair 0 upfront (weights already in flight)
                for step in proj_steps(0, wq0_sb, wk0_sb):
                    step()

                from collections import deque

                work = deque()

                def out_proj_steps(qh):
                    """Output projection for q rows qh*1024..qh*1024+1024:
                    8 q-tiles x (2ch x 4d) matmuls + ACT copy + DMA."""
                    state = {}
                    for qt_l in range(8):
                        qt_i = qh * 8 + qt_l

                        def o_alloc(qt_i=qt_i):
                            state["psf"] = psmain.tile(
                                [128, 1024], f32, tag="ps", name=f"pso{qt_i}"
                            )

                        yield o_alloc
                        for ch in range(2):
                            for d0 in range(0, NP, 2):

                                def o_mm(qt_i=qt_i, ch=ch, d0=d0):
                                    qs = slice(qt_i * 128, (qt_i + 1) * 128)
                                    for d in (d0, d0 + 1):
                                        nc.tensor.matmul(
                                            state["psf"][:, ch * 512 : (ch + 1) * 512],
                                            concat[d][:, qs],
                                            wot_sb[d][:, ch * 512 : (ch + 1) * 512],
                                            start=(d == 0),
                                            stop=(d == NP - 1),
                                        )

                                yield o_mm

                        def o_out(qt_i=qt_i):
                            o_sb = outp.tile([128, 1024], f32, tag="out")
                            nc.vector.tensor_copy(o_sb[:], state["psf"][:])
                            nc.sync.dma_start(
                                y[qt_i * 128 : (qt_i + 1) * 128, :], o_sb[:]
                            )

                        yield o_out

                # ---- attention: (qh, h, kv) with lag-1 PV + drip filler ----
                def emit_pv(p, lh, kv, ps_o, pm_t):
                    for ch in range(2):
                        chs = slice(ch * 512, (ch + 1) * 512)
                        nc.tensor.matmul(
                            ps_o[:, chs],
                            vaug(p, kv)[:, lh * 65 : (lh + 1) * 65],
                            pm_t[:, chs],
                            start=(kv == 0),
                            stop=(kv == NKV - 1),
                        )

                for qh in range(NQH):
                    if qh == 1:
                        work.extend(out_proj_steps(0))
                    for h in range(HL):
                        p, lh = h // 2, h % 2
                        if qh == 0 and h % 2 == 0 and h // 2 + 1 < NP:
                            work.extend(proj_steps(h // 2 + 1))
                        qt, kt = qt_all[p], kt_all[p]
                        hp = slice(lh * 64, (lh + 1) * 64)
                        ps_o = pso.tile([65, 1024], f32, tag="po", name=f"po{qh}_{h}")
                        prev_pm = None
                        for kv in range(NKV):
                            kvs = slice(kv * 128, (kv + 1) * 128)
                            ps_s = psmain.tile(
                                [128, 1024], f32, tag="ps", name=f"s{qh}_{h}_{kv}"
                            )
                            for ch in range(2):
                                cs = qh * 1024 + ch * 512
                                nc.tensor.matmul(
                                    ps_s[:, ch * 512 : (ch + 1) * 512],
                                    kt[hp, kvs],
                                    qt[hp, cs : cs + 512],
                                    start=True,
                                    stop=True,
                                )
                            if work:
                                work.popleft()()
                                if len(work) > 20 and work:
                                    work.popleft()()
                            pe_t = pexp.tile([128, 1024], bf16, tag="pe")
                            nc.scalar.activation(
                                pe_t[:], ps_s[:], ACT.Exp, scale=0.125
                            )
                            pm_t = pmask.tile([128, 1024], bf16, tag="pm")
                            nc.vector.tensor_mul(
                                pm_t[:], pe_t[:], msk_sb[(qh, kv)][:]
                            )
                            if qh == 0 and h == HL - 1:
                                # last qh0 reader of this mask tile: reload
                                # it with the qh1 slice (prefetch, WAR-safe)
                                load_msk(1, kv)
                            if prev_pm is not None:
                                emit_pv(p, lh, kv - 1, ps_o, prev_pm)
                            prev_pm = pm_t
                        emit_pv(p, lh, NKV - 1, ps_o, prev_pm)
                        # finalize (DVE/DMA/GPSIMD): copy PSUM out (frees the
                        # po slot), bounce the softmax sums through DRAM into
                        # [128, 8] so the reciprocal uses all DVE lanes, DMA
                        # the broadcast reciprocal back, multiply on DVE,
                        # bias-add on GPSIMD.
                        o_sb = finp.tile([65, 1024], f32, tag="osb", bufs=2)
                        nc.vector.tensor_copy(o_sb[:], ps_o[:])
                        dsum = dscr.tile([1024], f32, tag="dsum")
                        nc.sync.dma_start(
                            dsum.rearrange("(a b) -> a b", a=1), o_sb[64:65, :]
                        )
                        rs = finp.tile([128, 8], f32, tag="rs")
                        nc.sync.dma_start(rs[:], dsum.rearrange("(a b) -> a b", a=128))
                        rr = finp.tile([128, 8], f32, tag="rr")
                        nc.vector.reciprocal(rr[:], rs[:])
                        drec = dscr.tile([1024], f32, tag="drec")
                        nc.sync.dma_start(drec.rearrange("(a b) -> a b", a=128), rr[:])
                        rb = finp.tile([64, 1024], f32, tag="rb", bufs=2)
                        nc.sync.dma_start(
                            rb[:],
                            drec.rearrange("(a b) -> a b", a=1).partition_broadcast(64),
                        )
                        tmp = finp.tile([64, 1024], bf16, tag="tmp", bufs=2)
                        nc.vector.tensor_mul(tmp[:], o_sb[0:64, :], rb[:])
                        nc.vector.tensor_scalar_add(
                            concat[p][hp, qh * 1024 : (qh + 1) * 1024],
                            tmp[:],
                            bv_sb[:, h : h + 1],
                        )
                while work:
                    work.popleft()()
                # ---- tail: output projection for qh1 ----
                for step in out_proj_steps(1):
                    step()

    _split_excess_waits(nc, max_waits=1)
    return nc


def _prep_inputs(context_sequence, value_sequence, mask, Wq, bq, Wk, bk, Wv, bv, Wo, bo):
    """Host-side shard prep: slice/transpose/cast per core."""
    ctx = np.asarray(context_sequence, dtype=np.float32)
    val = np.asarray(value_sequence, dtype=np.float32)
    mask = np.asarray(mask)
    Wq = np.asarray(Wq, dtype=np.float32)
    Wk = np.asarray(Wk, dtype=np.float32)
    Wv = np.asarray(Wv, dtype=np.float32)
    Wo = np.asarray(Wo, dtype=np.float32)
    bq = np.asarray(bq, dtype=np.float32)
    bk = np.asarray(bk, dtype=np.float32)
    bv = np.asarray(bv, dtype=np.float32)

    mskt = np.ascontiguousarray((mask == 0).T).astype(BF16)  # [S, S], 1.0=keep
    wo_t = np.ascontiguousarray(Wo.T)  # [din, dout]

    # per-batch transposed activations
    xt_b, xtv_b = [], []
    for b in range(B):
        xt_b.append(np.ascontiguousarray(ctx[b].T).astype(BF16))  # [D, S]
        xv = np.ascontiguousarray(val[b].T).astype(BF16)
        xtv_b.append(
            np.ascontiguousarray(
                xv.reshape(NDT, 128, 4, 512).transpose(2, 0, 1, 3)
            )
        )  # [4 sc, 8 d, 128, 512]

    # per-head-half weights
    def whalf(W, hh):  # [H, D, DK] -> [NP, NDT, 128, 128]
        Wf = W[hh * HL : (hh + 1) * HL].transpose(1, 0, 2).reshape(D, HL * DK)
        return np.ascontiguousarray(
            Wf.reshape(NDT, 128, NP, 128).transpose(2, 0, 1, 3)
        ).astype(BF16)

    shard = []
    for hh in range(2):
        g0 = hh * HL
        wq_t = whalf(Wq, hh)
        wk_t = whalf(Wk, hh)
        wv_t = np.ascontiguousarray(
            Wv[g0 : g0 + HL].transpose(1, 0, 2).reshape(D, HL * DK).reshape(
                NDT, 128, HL * DK
            )
        ).astype(BF16)
        wot_t = np.ascontiguousarray(
            wo_t[hh * 512 : (hh + 1) * 512, :].reshape(NP, 128, D)
        ).astype(BF16)
        bq_tt = np.ascontiguousarray(bq[g0 : g0 + HL].reshape(NP, 128).T)
        bk_tt = np.ascontiguousarray(bk[g0 : g0 + HL].reshape(NP, 128).T)
        bv_tt = np.ascontiguousarray(bv[g0 : g0 + HL].reshape(HL, DK).T)
        shard.append((wq_t, wk_t, wv_t, wot_t, bq_tt, bk_tt, bv_tt))

    in_maps = []
    for c in range(NCORES):
        b, hh = c // 2, c % 2
        wq_t, wk_t, wv_t, wot_t, bq_tt, bk_tt, bv_tt = shard[hh]
        in_maps.append(
            {
                "xt": xt_b[b],
                "xtv": xtv_b[b],
                "mskt": mskt,
                "wq": wq_t,
                "wk": wk_t,
                "wv": wv_t,
                "wot": wot_t,
                "bq_t": bq_tt,
                "bk_t": bk_tt,
                "bv_t": bv_tt,
            }
        )
    return in_maps


def _execute(inputs, trace=False):
    from concourse.bass_utils import run_bass_kernel_spmd

    if "nc" not in _CACHE:
        _CACHE["nc"] = _build()
    nc = _CACHE["nc"]
    in_maps = _prep_inputs(**inputs)
    res = run_bass_kernel_spmd(nc, in_maps, list(range(NCORES)), trace=trace)
    bo = np.asarray(inputs["bo"], dtype=np.float32)
    out = np.empty((B, S, D), dtype=np.float32)
    for b in range(B):
        out[b] = res.results[2 * b]["y"] + res.results[2 * b + 1]["y"] + bo[None, :]
    return out, res.exec_time_ns


def kernel(**inputs):
    out, _ = _execute(inputs, trace=False)
    return out
